# revision 6
# baseline (speedup 1.0000x reference)
"""3-layer GCN (GCNConv x3 + LeakyReLU, PyG semantics) on 8 Trainium2 cores.

Strategy (graph-parallel over destination nodes):
  - Nodes are partitioned into 8 contiguous ranges; core c owns range c and
    computes the output rows for its own nodes.
  - Per layer: G = dinv * (H @ W) computed for own nodes (dense phase), then
    AllGather of G so each core holds the full node-feature table in its
    DRAM, then dma_gather over dst-sorted edges + one-hot matmul segment-sum
    into PSUM per 128-dst tile, then epilogue
        H' = lrelu(dinv * gathered_sum + dinv^2 * (H@W) + bias)
    (the dinv^2 term is the self-loop; symmetric normalization
    dinv[s]*dinv[d] is factored into table pre-scale + per-dst post-scale,
    so no per-edge scaling is needed).
  - dma_gather indices are int16, so the gather table (100352 rows) is
    addressed through 4 sub-table views of 25088 rows; edges are bucketed
    by source range.

The Bass program is SPMD: one program, per-core input data. Chunk counts
per (tile, bucket) are shared across cores (max over cores, padded with
dummy edges whose one-hot column is zero: dstrel = -1).
"""
import sys

sys.path.insert(0, "/opt/trn_rl_repo")

import numpy as np

import concourse.bacc as bacc
import concourse.mybir as mybir
import concourse.tile as tile
from concourse import library_config
from concourse.bass_utils import run_bass_kernel_spmd
from concourse.masks import make_identity

_F32 = mybir.dt.float32
_I16 = mybir.dt.int16
P = 128
D = 64
NEG_SLOPE = 0.01


class Cfg:
    def __init__(self, n_nodes=100000, cores=8, group=6, layers=3):
        self.N = n_nodes
        self.CORES = cores
        self.NPC = self.N // cores            # nodes owned per core
        self.TILES = (self.NPC + P - 1) // P  # dst tiles per core
        self.RPC = self.TILES * P             # padded rows per core
        self.GR = cores * self.RPC            # gather-table rows
        self.NBUCK = max(1, -(-self.GR // 25088))
        assert self.GR % self.NBUCK == 0
        self.BUCK_ROWS = self.GR // self.NBUCK
        assert self.BUCK_ROWS <= 32767 or self.GR <= 32767
        self.GROUP = group                    # dst tiles per gather group
        self.LAYERS = layers


DEFAULT_CFG = Cfg()


def _preprocess(edge_index, cfg):
    """Sort/bucket/pad edges; build per-core device arrays and metadata."""
    src = np.asarray(edge_index[0], dtype=np.int64)
    dst = np.asarray(edge_index[1], dtype=np.int64)
    N, CORES, NPC, RPC = cfg.N, cfg.CORES, cfg.NPC, cfg.RPC
    TILES, NBUCK, BUCK_ROWS = cfg.TILES, cfg.NBUCK, cfg.BUCK_ROWS

    deg = np.bincount(dst, minlength=N).astype(np.float32) + 1.0  # + self loop
    dinv = (1.0 / np.sqrt(deg)).astype(np.float32)

    gidx = (src // NPC) * RPC + (src % NPC)   # gather-table row of source
    bucket = gidx // BUCK_ROWS
    lidx = gidx % BUCK_ROWS

    owner = dst // NPC
    dloc = dst - owner * NPC
    tile_id = dloc // P
    dstrel = dloc % P

    counts = np.zeros((CORES, TILES, NBUCK), dtype=np.int64)
    np.add.at(counts, (owner, tile_id, bucket), 1)
    order = np.lexsort((bucket, tile_id, owner))
    sl = lidx[order]
    sr = dstrel[order]

    k_shared = (counts.max(axis=0) + P - 1) // P  # [TILES, NBUCK] chunks
    k_pad = k_shared * P

    groups = [list(range(g, min(g + cfg.GROUP, TILES)))
              for g in range(0, TILES, cfg.GROUP)]

    # flat layout: group -> bucket -> tile -> chunks (so each (group,
    # bucket) span is contiguous: one dma_gather per (group, bucket))
    off = {}
    pos = 0
    for grp in groups:
        for b in range(NBUCK):
            for t in grp:
                off[(t, b)] = pos
                pos += int(k_pad[t, b])
    tot_idx = pos
    tot_ch = tot_idx // P

    # boundaries of each core's (t, b) section in the sorted edge list
    cum = np.zeros((CORES, TILES, NBUCK + 1), dtype=np.int64)
    cum[:, :, 1:] = np.cumsum(counts, axis=2)
    flat_counts = counts.sum(axis=2)            # [CORES, TILES]
    run = np.cumsum(flat_counts.reshape(-1))
    base = np.zeros(CORES * TILES, dtype=np.int64)
    base[1:] = run[:-1]
    base = base.reshape(CORES, TILES)

    per_core = []
    for c in range(CORES):
        lidx_flat = np.zeros(tot_idx, dtype=np.int16)
        drel_flat = np.full(tot_idx, -1.0, dtype=np.float32)
        for t in range(TILES):
            for b in range(NBUCK):
                n = int(counts[c, t, b])
                if n == 0:
                    continue
                s0 = int(base[c, t] + cum[c, t, b])
                o = off[(t, b)]
                lidx_flat[o:o + n] = sl[s0:s0 + n].astype(np.int16)
                drel_flat[o:o + n] = sr[s0:s0 + n].astype(np.float32)
        idx16 = np.tile(lidx_flat.reshape(tot_idx // 16, 16).T, (8, 1)).copy()
        drel = drel_flat.reshape(tot_ch, P).T.copy()
        per_core.append({"idx16": idx16, "dstrel": drel})

    meta = {
        "k_shared": k_shared,
        "groups": groups,
        "off": off,
        "tot_idx": tot_idx,
        "tot_ch": tot_ch,
        "dinv": dinv,
    }
    return meta, per_core


def _build_program(meta, cfg):
    k_shared = meta["k_shared"]
    groups = meta["groups"]
    off = meta["off"]
    tot_idx = meta["tot_idx"]
    CORES, TILES, RPC, GR = cfg.CORES, cfg.TILES, cfg.RPC, cfg.GR
    NBUCK, BUCK_ROWS = cfg.NBUCK, cfg.BUCK_ROWS

    grp_info = []
    for grp in groups:
        i0 = off[(grp[0], 0)]
        blocks = sum(int(k_shared[t, b]) for t in grp for b in range(NBUCK))
        boff = {k: (off[k] - i0) // P for k in off if k[0] in grp}
        bko = {}
        for b in range(NBUCK):
            ts = [t for t in grp if k_shared[t, b] > 0]
            if ts:
                kb = sum(int(k_shared[t, b]) for t in grp)
                bko[b] = (boff[(ts[0], b)], kb)
        grp_info.append({"tiles": grp, "blocks": blocks, "boff": boff,
                         "i0": i0, "bko": bko})
    gblk_max = max(g["blocks"] for g in grp_info)
    kmaxb = int(k_shared.max())

    nc = bacc.Bacc("TRN2", debug=False)
    nc.num_devices = CORES

    xT_in = nc.dram_tensor("xT", [D, RPC], _F32, kind="ExternalInput")
    dinv1_in = nc.dram_tensor("dinv1", [P, TILES], _F32, kind="ExternalInput")
    dinv2_in = nc.dram_tensor("dinv2", [P, TILES], _F32, kind="ExternalInput")
    w_in = [nc.dram_tensor(f"W{i + 1}", [D, D], _F32, kind="ExternalInput")
            for i in range(3)]
    bias_in = [nc.dram_tensor(f"bias{i + 1}", [P, D], _F32,
                              kind="ExternalInput") for i in range(3)]
    iota_in = nc.dram_tensor("iota", [P, P], _F32, kind="ExternalInput")
    idx_in = nc.dram_tensor("idx16", [P, tot_idx // 16], _I16,
                            kind="ExternalInput")
    drel_in = nc.dram_tensor("dstrel", [P, meta["tot_ch"]], _F32,
                             kind="ExternalInput")
    out_t = nc.dram_tensor("out", [RPC, D], _F32, kind="ExternalOutput")

    with tile.TileContext(nc) as tc:
        with tc.tile_pool(name="dram", bufs=1, space="DRAM") as dram, \
             tc.tile_pool(name="const", bufs=1) as cst, \
             tc.tile_pool(name="persist", bufs=1) as per, \
             tc.tile_pool(name="msgp", bufs=2) as msgp, \
             tc.tile_pool(name="idxp", bufs=2) as idxp, \
             tc.tile_pool(name="qp", bufs=4) as qp, \
             tc.tile_pool(name="wk", bufs=3) as wk, \
             tc.tile_pool(name="psa", bufs=2, space="PSUM") as psa, \
             tc.tile_pool(name="psg", bufs=2, space="PSUM") as psg, \
             tc.tile_pool(name="pst", bufs=2, space="PSUM") as pst:

            nc.gpsimd.load_library(library_config.mlp)

            g_owns = [dram.tile([RPC, D], _F32, name=f"g_own{i}")
                      for i in range(3)]
            g_fulls = [dram.tile([GR, D], _F32, addr_space="Shared",
                                 name=f"g_full{i}") for i in range(3)]

            iota = cst.tile([P, P], _F32)
            nc.sync.dma_start(iota[:], iota_in[:])
            ident = cst.tile([P, P], _F32)
            make_identity(nc, ident[:])
            dinv1 = cst.tile([P, TILES], _F32)
            nc.sync.dma_start(dinv1[:], dinv1_in[:])
            dinv2 = cst.tile([P, TILES], _F32)
            nc.sync.dma_start(dinv2[:], dinv2_in[:])
            ws, bs = [], []
            for i in range(3):
                w = cst.tile([D, D], _F32, name=f"w{i}")
                nc.sync.dma_start(w[:], w_in[i][:])
                ws.append(w)
                bt = cst.tile([P, D], _F32, name=f"b{i}")
                nc.sync.dma_start(bt[:], bias_in[i][:])
                bs.append(bt)

            ht = per.tile([D, RPC], _F32)          # H.T (current layer input)
            nc.sync.dma_start(ht[:], xT_in[:])
            g2b = per.tile([P, TILES * D], _F32)   # dinv^2*(H@W) + bias

            for L in range(cfg.LAYERS):
                # ---------- phase A: G = dinv * (H @ W) ----------
                g_own = g_owns[L]
                g_full = g_fulls[L]
                for t in range(TILES):
                    pg = psg.tile([P, D], _F32, tag="pg", name=f"pg{L}_{t}")
                    nc.tensor.matmul(
                        pg[:], lhsT=ht[:, t * P:(t + 1) * P], rhs=ws[L % 3][:],
                        start=True, stop=True,
                    )
                    g = wk.tile([P, D], _F32, tag="g", name=f"g{L}_{t}")
                    nc.vector.tensor_scalar_mul(g[:], pg[:], dinv1[:, t:t + 1])
                    nc.sync.dma_start(g_own[t * P:(t + 1) * P, :], g[:])
                    g2 = wk.tile([P, D], _F32, tag="g2", name=f"g2_{L}_{t}")
                    nc.vector.tensor_scalar_mul(g2[:], pg[:],
                                                dinv2[:, t:t + 1])
                    nc.vector.tensor_tensor(
                        out=g2b[:, t * D:(t + 1) * D], in0=g2[:], in1=bs[L % 3][:],
                        op=mybir.AluOpType.add,
                    )

                # ---------- phase B: AllGather ----------
                nc.gpsimd.collective_compute(
                    "AllGather",
                    mybir.AluOpType.bypass,
                    replica_groups=[list(range(CORES))],
                    ins=[g_own[:]],
                    outs=[g_full[:]],
                )

                # ---------- phase C: edge aggregation ----------
                for gi, gf in enumerate(grp_info):
                    blocks = gf["blocks"]
                    i0 = gf["i0"]
                    nidx_g = blocks * P
                    msg = msgp.tile([P, gblk_max, D], _F32, tag="msg",
                                    name=f"msg{L}_{gi}",
                                    padded_shape=[P, gblk_max, D])
                    idx_sb = idxp.tile([P, (gblk_max * P) // 16], _I16,
                                       tag="idx", name=f"idx{L}_{gi}",
                                       padded_shape=[P, (gblk_max * P) // 16])
                    drel_sb = idxp.tile([P, gblk_max], _F32, tag="drel",
                                        name=f"drel{L}_{gi}",
                                        padded_shape=[P, gblk_max])
                    nc.sync.dma_start(
                        idx_sb[:, :nidx_g // 16],
                        idx_in[:, i0 // 16:(i0 + nidx_g) // 16],
                    )
                    nc.sync.dma_start(
                        drel_sb[:, :blocks],
                        drel_in[:, i0 // P:i0 // P + blocks],
                    )
                    for b, (bo, kb) in gf["bko"].items():
                        # dma_gather fails on HW above 1024 indices per call
                        for s0 in range(0, kb, 8):
                            kk = min(8, kb - s0)
                            bo2 = bo + s0
                            nidx = kk * P
                            nc.gpsimd.dma_gather(
                                msg[:, bo2:bo2 + kk, :],
                                g_full[b * BUCK_ROWS:(b + 1) * BUCK_ROWS, :],
                                idx_sb[:, bo2 * P // 16:
                                       (bo2 * P + nidx) // 16],
                                nidx, nidx, D,
                            )

                    for t in gf["tiles"]:
                        kt = int(k_shared[t].sum())
                        pa = psa.tile([P, D], _F32, tag="pa", name=f"pa{L}_{t}")
                        lastb = _last_b(k_shared, t, NBUCK)
                        first = True
                        for b in range(NBUCK):
                            k = int(k_shared[t, b])
                            if k == 0:
                                continue
                            bo = gf["boff"][(t, b)]
                            q = qp.tile([P, k, P], _F32, tag="q",
                                        name=f"q{L}_{t}_{b}",
                                        padded_shape=[P, kmaxb, P])
                            nc.vector.tensor_tensor(
                                out=q[:],
                                in0=iota[:].rearrange("p (c f) -> p c f", c=1)
                                    .to_broadcast([P, k, P]),
                                in1=drel_sb[:, bo:bo + k]
                                    .rearrange("p (c f) -> p c f", f=1)
                                    .to_broadcast([P, k, P]),
                                op=mybir.AluOpType.is_equal,
                            )
                            for j in range(k):
                                nc.tensor.matmul(
                                    pa[:], lhsT=q[:, j, :],
                                    rhs=msg[:, bo + j, :],
                                    start=first,
                                    stop=(b == lastb and j == k - 1),
                                )
                                first = False
                        v = wk.tile([P, D], _F32, tag="v", name=f"v{L}_{t}")
                        if kt > 0:
                            nc.vector.tensor_scalar_mul(
                                v[:], pa[:], dinv1[:, t:t + 1])
                            nc.vector.tensor_tensor(
                                out=v[:], in0=v[:],
                                in1=g2b[:, t * D:(t + 1) * D],
                                op=mybir.AluOpType.add)
                        else:
                            nc.vector.tensor_copy(
                                v[:], g2b[:, t * D:(t + 1) * D])
                        hm = wk.tile([P, D], _F32, tag="hm", name=f"hm{L}_{t}")
                        nc.vector.tensor_scalar_mul(hm[:], v[:], NEG_SLOPE)
                        h = wk.tile([P, D], _F32, tag="h", name=f"h{L}_{t}")
                        nc.vector.tensor_tensor(
                            out=h[:], in0=hm[:], in1=v[:],
                            op=mybir.AluOpType.max)
                        if L < cfg.LAYERS - 1:
                            pt = pst.tile([D, P], _F32, tag="pt",
                                          name=f"pt{L}_{t}")
                            nc.tensor.transpose(pt[:], h[:], ident[:])
                            nc.vector.tensor_copy(ht[:, t * P:(t + 1) * P],
                                                  pt[:])
                        else:
                            nc.sync.dma_start(out_t[t * P:(t + 1) * P, :],
                                              h[:])

    nc.compile()
    return nc


def _last_b(k_shared, t, nbuck):
    lb = 0
    for b in range(nbuck):
        if k_shared[t, b] > 0:
            lb = b
    return lb


def make_in_maps(x, Ws, bss, meta, per_core, cfg):
    dinv = meta["dinv"]
    CORES, NPC, RPC, TILES = cfg.CORES, cfg.NPC, cfg.RPC, cfg.TILES
    iota_np = np.broadcast_to(np.arange(P, dtype=np.float32), (P, P)).copy()
    in_maps = []
    for c in range(CORES):
        sl = slice(c * NPC, (c + 1) * NPC)
        xT = np.zeros((D, RPC), np.float32)
        xT[:, :NPC] = x[sl].T
        d1c = np.zeros(RPC, np.float32)
        d1c[:NPC] = dinv[sl]
        d1 = d1c.reshape(TILES, P).T.copy()
        d2 = (d1 * d1).astype(np.float32)
        im = {
            "xT": xT,
            "dinv1": d1,
            "dinv2": d2,
            "iota": iota_np,
            "idx16": per_core[c]["idx16"],
            "dstrel": per_core[c]["dstrel"],
        }
        for i in range(3):
            im[f"W{i + 1}"] = Ws[i]
            im[f"bias{i + 1}"] = np.broadcast_to(
                bss[i], (P, D)).astype(np.float32).copy()
        in_maps.append(im)
    return in_maps


_CACHE = {}


def kernel(x, edge_index, W1, b1, W2, b2, W3, b3):
    cfg = DEFAULT_CFG
    x = np.asarray(x, dtype=np.float32)
    Ws = [np.asarray(w, dtype=np.float32) for w in (W1, W2, W3)]
    bss = [np.asarray(b, dtype=np.float32) for b in (b1, b2, b3)]

    ei = np.asarray(edge_index)
    key = hash(ei[:, ::997].tobytes()) ^ hash(ei.shape)
    if key not in _CACHE:
        meta, per_core = _preprocess(ei, cfg)
        nc = _build_program(meta, cfg)
        _CACHE[key] = (meta, per_core, nc)
    meta, per_core, nc = _CACHE[key]

    in_maps = make_in_maps(x, Ws, bss, meta, per_core, cfg)
    res = run_bass_kernel_spmd(nc, in_maps, core_ids=list(range(cfg.CORES)))
    out = np.empty((cfg.N, D), np.float32)
    for c in range(cfg.CORES):
        out[c * cfg.NPC:(c + 1) * cfg.NPC] = res.results[c]["out"][:cfg.NPC]
    return out
